# revision 31
# baseline (speedup 1.0000x reference)
"""Depthwise causal conv1d (W=8) with 3 interleaved weight sets, on 8 TRN2 cores.

Reference computes r/o/a = depthwise_causal_conv(x, {rtg,obs,act}_{w,b}) and
interleaves out[:, t] = {r,o,a}[:, t] by t % 3.  Only the t%3-matching third of
each conv is needed, so total work is exactly one conv: for each output t,
out[b,t,h] = sum_k x[b, t-7+k, h] * w_{t%3}[h, k] + b_{t%3}[h].

Strategy (channel-parallel, 96 channels per core, banded-Toeplitz matmul,
two channels packed per stationary matrix):
  - time goes on the PE contraction axis.  For channel pair (ca, cb), one
    [112 x 96] fp16 stationary matrix is block-diagonal: rows 0..54 hold
    ca's banded Toeplitz T[m, p] = w_{p%3}[ca, m-p] (0 <= m-p < 8) over
    output columns 0..47, rows 55..109 hold cb's band over columns 48..95.
    S=48 outputs per block (48 % 3 == 0 keeps the t%3 phase pattern the
    same in every block; window V = S+7 = 55 rows; 64 blocks cover T=3072
    exactly).  Rows 110/111 of the moving tensor are a constant 1.0 and
    the matching stationary rows hold the per-channel bias columns,
    folding the bias into the matmul.
  - rhs = [112 x 1024] stacks both channels' input windows for all
    1024 = 16 batches x 64 blocks columns, so one weight load serves a
    pair's whole workload (halves the dense-w DMA vs 1 ch/stationary and
    computes 8 useful MACs per PE column-row vs 1 for a diag formulation).
  - each pair runs 2 matmuls (columns 0:512, 512:1024 — one f32 PSUM bank
    each) that share one ldweights; a post-compile pass drops the
    redundant second weight load.
  - per-pair 2-bank PSUM tiles in a 4-deep ring; per pair ONE ACT-or-DVE
    op downcast-evicts [96 x 2x512] to fp16 into a per-engine out tile:
    single-writer-engine tiles avoid a measured cross-engine
    write-after-write serialization that stalls PSUM reuse.
  - in-DMAs dispatched from the SP HWDGE, out-DMAs from the ACT HWDGE
    (out-DMAs block on evictions; mixing them into the SP queues
    head-of-line blocks the always-ready in-DMAs - measured 2x slowdown);
    big contiguous per-partition rows spread across all 16 DMA engines
    (~23 GB/s each), which is the binding resource for this kernel.
  - host pre/post stages the overlapped-window layout (fp16, unit-stride).
fp16 end-to-end rel err ~6e-4.
"""

import os
import numpy as np
from numpy.lib.stride_tricks import as_strided

B, T, H, W = 16, 3072, 768, 8
NCORES = 8
HC = H // NCORES             # 96 channels per core
S = 48                       # outputs per block (multiple of 3; 48*64 == T)
V = S + W - 1                # 55-row window per channel
NB = T // S                  # 64 blocks, exact cover
PADL = W - 1                 # causal left zero-pad
XLEN = S * (NB - 1) + V      # 3079 padded time extent (= T + PADL)
COLS = B * NB                # 1024 rhs columns per channel
HB = COLS // 2               # 512 matmul column half (= one f32 PSUM bank)
SP2 = 2 * S                  # 96 output partitions per pair
KR = 2 * V + 2               # 112 contraction rows (2 windows + 2 bias rows)
BR = 2 * V                   # bias rows at 110 (ch_a) / 111 (ch_b)
NPAIR = HC // 2              # 48 channel pairs per core
PIT = 4                      # pairs per pipeline iteration
NIT = NPAIR // PIT           # 12 iterations
XPAD = 64                    # x row pad (elems): 8320B rows, not 8192
WCH = 4                      # iterations per stationary-weight DMA chunk
# Crucial DMA properties (measured):
#  - a single transfer's descriptors (one per partition row) spread over
#    `largest divisor of row-count <= 16` DMA engines.  KR=112 (16x7) and
#    SP2=96 (16x6) use all 16 engines; 124 rows degenerates to 4 engines
#    (2x kernel slowdown) and 108 rows to 12.
#  - 8192B (power-of-two) descriptor rows stream at ~19 B/ns/engine vs
#    ~23.4 for 7296B: hence the 128B x-row pad to break the alignment.

_cache = {}


def _dedupe_ldweights(nc):
    """bacc lowers every 16-bit matmul to an InstLdweights + InstMatmult pair.
    The PE serializes each load (~200ns) before its matmul.  The two
    half-column matmuls of a pair share the same stationary matrix, so drop
    the redundant reload: remove an InstLdweights whose weights AP equals the
    previous one on the PE stream, carrying its semaphore waits onto the next
    PE instruction.  The 64B ISA word has one wait slot, so only dedupe when
    the waits fit."""
    import concourse.mybir as mybir

    removed = 0
    for fn in nc.m.functions:
        for blk in fn.blocks:
            insts = list(blk.instructions)
            drop = set()
            last_key = None
            for i, inst in enumerate(insts):
                if getattr(inst, "engine", None) != mybir.EngineType.PE:
                    continue
                tn = type(inst).__name__
                if tn == "InstLdweights":
                    a = inst.ins[0]
                    key = (a.memref, a.offset, str(a.ap), str(a.dtype))
                    si = inst.sync_info
                    my_waits = list(si.on_wait) if si is not None else []
                    has_upd = si is not None and len(si.on_update) > 0
                    if key == last_key and not has_upd:
                        nxt = None
                        for j in range(i + 1, len(insts)):
                            if getattr(insts[j], "engine", None) == mybir.EngineType.PE:
                                nxt = insts[j]
                                break
                        if nxt is not None:
                            nsi = nxt.sync_info
                            n_waits = len(nsi.on_wait) if nsi is not None else 0
                            if n_waits + len(my_waits) <= 1:
                                if my_waits:
                                    if nsi is None:
                                        nxt.sync_info = mybir.SyncInfo(
                                            on_wait=my_waits, on_update=[]
                                        )
                                    else:
                                        nsi.on_wait = list(nsi.on_wait) + my_waits
                                drop.add(i)
                                removed += 1
                                continue
                    last_key = key
                elif tn == "InstMatmult":
                    pass  # non-self-loading; PE array state unchanged
                else:
                    last_key = None  # be conservative about other PE ops
            if drop:
                blk.instructions = [x for i, x in enumerate(insts) if i not in drop]
    return removed


def _build_nc():
    import concourse.bacc as bacc
    import concourse.mybir as mybir
    import concourse.tile as tile

    nc = bacc.Bacc("TRN2", target_bir_lowering=False, debug=False)
    f32 = mybir.dt.float32
    f16 = mybir.dt.float16

    XROW = PIT * COLS + XPAD
    x_d = nc.dram_tensor("x", [NIT, KR, XROW], f16, kind="ExternalInput").ap()
    w_d = nc.dram_tensor("w", [NIT // WCH, KR, WCH * PIT * SP2], f16,
                         kind="ExternalInput").ap()
    y_d = nc.dram_tensor("y", [NIT, SP2, PIT * COLS], f16, kind="ExternalOutput").ap()

    with tile.TileContext(nc) as tc:
        with (
            tc.tile_pool(name="wp", bufs=3) as wp,
            tc.tile_pool(name="xp", bufs=5) as xp,
            tc.tile_pool(name="op", bufs=6) as op_,
            tc.tile_pool(name="ps", bufs=4, space="PSUM") as psp,
        ):
            HP2 = PIT // 2
            HXE = PIT * COLS // 2
            WHE = WCH * PIT * SP2 // 2
            wt = None
            for it in range(NIT):
                if it % WCH == 0:
                    wt = wp.tile([KR, WCH, PIT, SP2], f16, tag="w")
                    nc.sync.dma_start(wt[:], w_d[it // WCH])
                xt = xp.tile([KR, XROW], f16, tag="x")
                if it == 0:
                    # first pair's x ahead of the rest, so matmul 0 starts
                    # after ~560 KB instead of the full first-iteration load
                    nc.sync.dma_start(xt[:, 0:COLS], x_d[it][:, 0:COLS])
                    nc.sync.dma_start(xt[:, COLS:], x_d[it][:, COLS:])
                else:
                    nc.sync.dma_start(xt[:], x_d[it])
                # one 2-bank PSUM tile per pair (ring of 4) so eviction and
                # PSUM reuse proceed per-pair instead of per-iteration
                pss = []
                for q in range(PIT):
                    ps = psp.tile([SP2, 2, 512], f32, tag="ps")
                    for j in range(2):
                        nc.tensor.matmul(
                            ps[:, j, 0:HB],
                            wt[:, it % WCH, q],
                            xt[:, q * COLS + j * HB : q * COLS + (j + 1) * HB],
                            start=True, stop=True,
                        )
                    pss.append(ps)
                # evictions: pairs 0,1 on DVE -> ot_a; pairs 2,3 on ACT ->
                # ot_b.  Single-writer-engine tiles avoid the cross-engine
                # write-after-write serialization that stalls PSUM reuse.
                ota = op_.tile([SP2, HP2, 2, HB], f16, tag="oa")
                otb = op_.tile([SP2, HP2, 2, HB], f16, tag="ob")
                for q in range(PIT):
                    dst = ota[:, q] if q < HP2 else otb[:, q - HP2]
                    if q < HP2:
                        nc.vector.tensor_scalar_mul(dst, pss[q][:, :, 0:HB], 1.0)
                    else:
                        nc.scalar.copy(dst, pss[q][:, :, 0:HB])
                # all stores dispatch from the ACT HWDGE: out-DMAs block on
                # evictions, and mixing them into the SP queues head-of-line
                # blocks the always-ready in-DMAs (measured 2x regression)
                if it >= NIT - 3:
                    # per-pair stores at the end shorten the drain
                    for q in range(HP2):
                        nc.scalar.dma_start(
                            y_d[it][:, q * COLS : (q + 1) * COLS], ota[:, q])
                        nc.scalar.dma_start(
                            y_d[it][:, (HP2 + q) * COLS : (HP2 + q + 1) * COLS],
                            otb[:, q])
                else:
                    nc.scalar.dma_start(y_d[it][:, : HP2 * COLS], ota[:])
                    nc.scalar.dma_start(y_d[it][:, HP2 * COLS :], otb[:])

    nc.compile()
    n = _dedupe_ldweights(nc)
    if os.environ.get("KERNEL_VERBOSE"):
        print(f"deduped {n} ldweights")
    return nc


def _get_nc():
    if "nc" not in _cache:
        _cache["nc"] = _build_nc()
    return _cache["nc"]


def _install_ntff_hook():
    """antenv.axon_hooks is not shipped in this container; shim it so
    bass_utils can find the NTFF profile hook (trace=True path)."""
    import sys, types
    if "antenv.axon_hooks" in sys.modules:
        return
    mod = types.ModuleType("antenv.axon_hooks")
    mod._hook = None
    mod.set_axon_ntff_profile_hook = lambda h: setattr(mod, "_hook", h)
    mod.get_axon_ntff_profile_hook = lambda: mod._hook
    sys.modules["antenv.axon_hooks"] = mod
    try:
        from trn_agent_boot.trn_boot import _ntff_profile_via_ctypes
        mod._hook = _ntff_profile_via_ctypes("/opt/axon/libaxon_pjrt.so")
    except Exception:
        mod._hook = None


def kernel(x, rtg_w, rtg_b, obs_w, obs_b, act_w, act_b):
    from concourse import bass_utils

    x = np.asarray(x, dtype=np.float32)
    ws = np.stack([np.asarray(a, np.float32) for a in (rtg_w, obs_w, act_w)], 1)  # [H,3,W]
    bs = np.stack([np.asarray(a, np.float32) for a in (rtg_b, obs_b, act_b)], 1)  # [H,3]

    # staged input windows: xw[ch, m, (b, n)] = x[b, S*n + m - PADL, ch]
    xT = np.ascontiguousarray(x.transpose(2, 0, 1)).astype(np.float16)  # [H,B,T]
    xpad = np.zeros((H, B, XLEN), np.float16)
    xpad[:, :, PADL : PADL + T] = xT
    st = xpad.strides
    xw = as_strided(xpad, (H, B, NB, V), (st[0], st[1], S * st[2], st[2]))
    xw = np.ascontiguousarray(xw.transpose(0, 3, 1, 2)).reshape(H, V, COLS)

    # per-channel banded Toeplitz [V, S]: lh[ch, m, p] = w_{p%3}[ch, m-p]
    pidx = np.arange(S)
    lh = np.zeros((H, V, S), np.float32)
    for k in range(W):
        lh[:, pidx + k, pidx] = ws[:, pidx % 3, k]
    lh = lh.astype(np.float16)
    bcol = bs[:, pidx % 3].astype(np.float16)               # [H, S]

    # pair block-diagonal stationaries [H/2, 112, 96] with bias rows 110/111
    HP = H // 2
    wpair = np.zeros((HP, KR, SP2), np.float16)
    wpair[:, :V, :S] = lh[0::2]
    wpair[:, V : 2 * V, S:] = lh[1::2]
    wpair[:, BR, :S] = bcol[0::2]
    wpair[:, BR + 1, S:] = bcol[1::2]
    # paired moving windows [H/2, 112, 1024] with the constant-1.0 bias rows
    xpair = np.zeros((HP, KR, COLS), np.float16)
    xpair[:, :V] = xw[0::2]
    xpair[:, V : 2 * V] = xw[1::2]
    xpair[:, BR : BR + 2] = 1.0

    in_maps = []
    for c in range(NCORES):
        p0 = c * NPAIR
        xg = xpair[p0 : p0 + NPAIR].reshape(NIT, PIT, KR, COLS)
        xc = np.zeros((NIT, KR, PIT * COLS + XPAD), np.float16)
        xc[:, :, : PIT * COLS] = xg.transpose(0, 2, 1, 3).reshape(NIT, KR, PIT * COLS)
        wc = wpair[p0 : p0 + NPAIR].reshape(NIT // WCH, WCH * PIT, KR, SP2)
        wc = np.ascontiguousarray(wc.transpose(0, 2, 1, 3))
        wc = wc.reshape(NIT // WCH, KR, WCH * PIT * SP2)
        in_maps.append({"x": xc, "w": wc})

    nc = _get_nc()
    trace = bool(int(os.environ.get("KERNEL_TRACE", "0")))
    if trace:
        _install_ntff_hook()
    res = bass_utils.run_bass_kernel_spmd(
        nc, in_maps, core_ids=list(range(NCORES)), trace=trace,
    )
    _cache["last_result"] = res

    out = np.empty((B, T, H), dtype=np.float32)
    for c in range(NCORES):
        y = res.results[c]["y"]                              # [NIT, SP2, PIT*COLS]
        y = y.reshape(NIT, 2, S, PIT, B, NB)                 # [it, half, p, q, b, n]
        y = y.transpose(4, 0, 3, 1, 5, 2)                    # [b, it, q, half, n, p]
        y = y.reshape(B, HC, NB * S)[:, :, :T]               # [b, ch, t]
        out[:, :, c * HC : (c + 1) * HC] = y.transpose(0, 2, 1).astype(np.float32)
    return out
